# revision 37
# baseline (speedup 1.0000x reference)
"""CrossViewAttention Trainium2 kernel.

Math: for each batch row b with features f1, f2 (D=1024):
  Q_s = f_s Wq^T + bq ; K_t = f_t Wk^T + bk ; V_t = f_t Wv^T + bv
  scores s_st = Q_s.K_t / sqrt(D); attn = softmax over t; out = sum_s attn_st V_t

2-way softmax collapses to sigmoids of score differences:
  d1 = (s11-s12) = (f1.(g @ M^T) + g.ck)/sqrt(D)
  d2 = (s21-s22) = (f2.(g @ M^T) + g.ck)/sqrt(D)
  with g = f1-f2, M = Wq^T Wk, ck = Wk^T bq  (bk and bq-cross terms cancel)
  w1 = sigmoid(d1)+sigmoid(d2); w2 = 2-w1
  out = (w1*f1 + w2*f2) @ Wv^T + 2*bv

Per 128-row chunk only TWO 128x1024x1024 matmuls are needed (scores + output).
The scores matmul feeds a sigmoid, so it tolerates fp8: it runs as e4m3
DoubleRow (contraction 256/instruction, ~1.8x faster).  The output matmul
runs in bf16.  M' = 32*M^T is pre-scaled (power of two) so its entries sit
in e4m3's sweet spot; the dot scale becomes 2^-10.

All three per-chunk input streams (f1 bf16 | f2 bf16 | g^T fp8) are packed
into one 5KB-per-partition DMA to cut descriptor-generation cost; the dots
run all-bf16 (2x DVE) off a bf16 copy of the scores.

Sharding: batch split across 8 cores (2048 rows each), weights replicated.
"""

import sys

for _p in ("/opt/trn_rl_repo",):
    if _p not in sys.path:
        sys.path.insert(0, _p)

import numpy as np
import ml_dtypes

import concourse.bacc as bacc
import concourse.mybir as mybir
import concourse.tile as tile

F32 = mybir.dt.float32
BF16 = mybir.dt.bfloat16
F8 = mybir.dt.float8e4
U8 = mybir.dt.uint8

NPBF16 = ml_dtypes.bfloat16
NPF8 = ml_dtypes.float8_e4m3

B = 16384
D = 1024
NCORES = 8
R = B // NCORES          # rows per core
CH = 128                 # chunk rows
KT = D // 128            # contraction k-tiles (8)
KP = KT // 2             # DoubleRow k-pair tiles (4)
PKB = 2 * D * 2 + D      # packed bytes/partition: f1|f2 bf16 + g^T fp8
DOT_SCALE = float(2.0 ** -10)   # 1/sqrt(D) * (1/32 from M' prescale)


def build(nc, n_chunks, repeats=1, hw_loop=None):
    # f1|f2 rows packed per partition (bf16), g^T fp8 pair layout
    pkf = nc.dram_tensor("pkf", [n_chunks, 128, 2 * D], BF16, kind="ExternalInput").ap()
    gtb = nc.dram_tensor("gtb", [n_chunks, 128, KP, 2, CH], F8, kind="ExternalInput").ap()
    gckb = nc.dram_tensor("gckb", [128, n_chunks], F32, kind="ExternalInput").ap()
    # M' = 32*M^T in fp8 DoubleRow pair layout [128, KP, 2, D]
    mtb = nc.dram_tensor("mtb", [128, KP, 2, D], F8, kind="ExternalInput").ap()
    # Wv^T in bf16 [128, KT, D]
    wvt = nc.dram_tensor("wvt", [128, KT, D], BF16, kind="ExternalInput").ap()
    bv2f = nc.dram_tensor("bv2f", [128, D], F32, kind="ExternalInput").ap()
    idn = nc.dram_tensor("idn", [128, 128], BF16, kind="ExternalInput").ap()
    out = nc.dram_tensor("out", [n_chunks * CH, D], F32, kind="ExternalOutput").ap()

    with tile.TileContext(nc) as tc:
        with (
            tc.tile_pool(name="wpool", bufs=1) as wpool,
            tc.tile_pool(name="io", bufs=4) as io,
            tc.tile_pool(name="work", bufs=4) as work,
            tc.tile_pool(name="small", bufs=4) as small,
            tc.tile_pool(name="ps_ud", bufs=1, space="PSUM") as ps_ud,
            tc.tile_pool(name="ps_xt", bufs=2, space="PSUM") as ps_xt,
            tc.tile_pool(name="ps_o", bufs=2, space="PSUM") as ps_o,
        ):
            # resident weights
            mt_sb = wpool.tile([128, KP, 2, D], F8)
            nc.sync.dma_start(mt_sb[:], mtb[:])
            wv_sb = wpool.tile([128, KT, D], BF16)
            nc.sync.dma_start(wv_sb[:], wvt[:])
            bv_sb = wpool.tile([128, D], F32)
            nc.sync.dma_start(bv_sb[:], bv2f[:])
            id_sb = wpool.tile([128, 128], BF16)
            nc.sync.dma_start(id_sb[:], idn[:])
            gck_sb = wpool.tile([128, n_chunks], F32)
            nc.sync.dma_start(gck_sb[:], gckb[:])

            # Software-pipelined: body ii emits the front half (load, fp8
            # scores matmul, sigmoid weights, X build) for chunk ii and the
            # back half (transpose, bf16 output matmul, store) for chunk
            # ii-OFF, so the PE never waits on the DVE chain of the same
            # chunk.
            import contextlib

            OFF = 2
            n_iters = n_chunks * repeats
            backlog = []
            loop_cm = (
                tc.For_i(0, hw_loop, 1)
                if hw_loop is not None
                else contextlib.nullcontext()
            )
            with loop_cm:
                body(
                    nc, n_chunks, n_iters, OFF, backlog,
                    pkf, gtb, gckb, out,
                    mt_sb, wv_sb, bv_sb, id_sb, gck_sb,
                    io, work, small, ps_ud, ps_xt, ps_o,
                )
    return out


def body(
    nc, n_chunks, n_iters, OFF, backlog,
    pkf, gtb, gckb, out,
    mt_sb, wv_sb, bv_sb, id_sb, gck_sb,
    io, work, small, ps_ud, ps_xt, ps_o,
):
    if True:
        if True:
            for ii in range(n_iters + OFF):
                if ii < n_iters:
                    i = ii % n_chunks
                    # ---- packed loads: f1|f2 (bf16) and g^T (fp8 pairs)
                    ft = io.tile([128, 2 * D], BF16, tag="ft")
                    nc.sync.dma_start(ft[:], pkf[i])
                    f1t = ft[:, 0:D]
                    f2t = ft[:, D : 2 * D]
                    gt = io.tile([128, KP, 2, CH], F8, tag="gt")
                    nc.sync.dma_start(gt[:], gtb[i])

                    # ---- mm1: Ud = g @ M'^T (e4m3 DoubleRow) -> psum
                    ud = ps_ud.tile([128, D], F32, tag="ud")
                    for j in range(KP):
                        lhs = gt[:, j]
                        st = j == 0
                        sp = j == KP - 1
                        nc.tensor.matmul(
                            ud[:, 0:512],
                            lhs,
                            mt_sb[:, j, :, 0:512],
                            start=st,
                            stop=sp,
                            perf_mode=mybir.MatmulPerfMode.DoubleRow,
                        )
                        nc.tensor.matmul(
                            ud[:, 512:1024],
                            lhs,
                            mt_sb[:, j, :, 512:1024],
                            start=st,
                            stop=sp,
                            perf_mode=mybir.MatmulPerfMode.DoubleRow,
                        )

                    # ---- us = bf16(Ud): frees the PSUM bank fast and lets
                    #      the dots run all-16-bit
                    us = work.tile([128, D], BF16, tag="us")
                    nc.scalar.copy(us[:], ud[:])

                    # ---- dots: d_s = sum(f_s * Ud)*2^-10 (g.ck/sqrt(D)
                    #      added later as the sigmoid's bias)
                    dd = small.tile([128, 2], F32, tag="dd")
                    scr1 = work.tile([128, D], BF16, tag="scr")
                    nc.vector.scalar_tensor_tensor(
                        out=scr1[:],
                        in0=f1t[:],
                        scalar=DOT_SCALE,
                        in1=us[:],
                        op0=mybir.AluOpType.mult,
                        op1=mybir.AluOpType.mult,
                        accum_out=dd[:, 0:1],
                    )
                    scr2 = work.tile([128, D], BF16, tag="scr")
                    nc.vector.scalar_tensor_tensor(
                        out=scr2[:],
                        in0=f2t[:],
                        scalar=DOT_SCALE,
                        in1=us[:],
                        op0=mybir.AluOpType.mult,
                        op1=mybir.AluOpType.mult,
                        accum_out=dd[:, 1:2],
                    )

                    # ---- w1 = sig(d1+gck)+sig(d2+gck) in ONE ACT op (the
                    #      per-partition accumulate does the s-sum); w2=2-w1
                    sg = small.tile([128, 2], F32, tag="sg")
                    w1 = small.tile([128, 1], F32, tag="w1")
                    nc.scalar.activation(
                        sg[:],
                        dd[:],
                        mybir.ActivationFunctionType.Sigmoid,
                        bias=gck_sb[:, i : i + 1],
                        accum_out=w1[:],
                    )
                    w2 = small.tile([128, 1], F32, tag="w2")
                    nc.scalar.activation(
                        w2[:],
                        w1[:],
                        mybir.ActivationFunctionType.Copy,
                        bias=2.0,
                        scale=-1.0,
                    )

                    # ---- X = w1*f1 + w2*f2 (bf16)
                    t1 = work.tile([128, D], BF16, tag="t1")
                    nc.vector.tensor_scalar(
                        t1[:], f2t[:], w2[:], None, op0=mybir.AluOpType.mult
                    )
                    xr = work.tile([128, D], BF16, tag="xr")
                    nc.vector.scalar_tensor_tensor(
                        out=xr[:],
                        in0=f1t[:],
                        scalar=w1[:],
                        in1=t1[:],
                        op0=mybir.AluOpType.mult,
                        op1=mybir.AluOpType.add,
                    )
                    backlog.append((i, xr))

                if ii >= OFF:
                    i0, xr0 = backlog.pop(0)
                    rs = i0 * CH
                    # ---- X^T via PE transpose (per 128-block), psum bf16
                    xt_ps = ps_xt.tile([128, D], BF16, tag="xt")
                    for k in range(KT):
                        nc.tensor.transpose(
                            xt_ps[:, k * 128 : (k + 1) * 128],
                            xr0[:, k * 128 : (k + 1) * 128],
                            id_sb[:],
                        )
                    xt = work.tile([128, D], BF16, tag="xts")
                    nc.scalar.copy(xt[:], xt_ps[:])

                    # ---- mm2: out = X @ Wv^T (bf16) -> psum
                    po = ps_o.tile([128, D], F32, tag="po")
                    for k in range(KT):
                        lhs = xt[:, k * 128 : (k + 1) * 128]
                        st = k == 0
                        sp = k == KT - 1
                        nc.tensor.matmul(
                            po[:, 0:512],
                            lhs,
                            wv_sb[:, k, 0:512],
                            start=st,
                            stop=sp,
                        )
                        nc.tensor.matmul(
                            po[:, 512:1024],
                            lhs,
                            wv_sb[:, k, 512:1024],
                            start=st,
                            stop=sp,
                        )

                    # ---- += 2bv (broadcast tile) and store
                    ob = work.tile([128, D], F32, tag="ob")
                    nc.vector.tensor_tensor(
                        ob[:], po[:], bv_sb[:], op=mybir.AluOpType.add
                    )
                    nc.sync.dma_start(out[rs : rs + CH, :], ob[:])

    return out


_CACHE = {}


def get_compiled(n_chunks=R // CH):
    key = n_chunks
    if key not in _CACHE:
        nc = bacc.Bacc(
            "TRN2", target_bir_lowering=False, debug=False, num_devices=NCORES
        )
        build(nc, n_chunks)
        nc.compile()
        _CACHE[key] = nc
    return _CACHE[key]


def prep_inputs(f1, f2, Wq, bq, Wk, bk, Wv, bv):
    """Host-side algebra + sharding. Returns per-core input maps."""
    f1 = np.ascontiguousarray(np.asarray(f1), dtype=np.float32)
    f2 = np.ascontiguousarray(np.asarray(f2), dtype=np.float32)
    Wq = np.asarray(Wq, dtype=np.float32)
    bq = np.asarray(bq, dtype=np.float32)
    Wk = np.asarray(Wk, dtype=np.float32)
    Wv = np.asarray(Wv, dtype=np.float32)
    bv = np.asarray(bv, dtype=np.float32)
    g = f1 - f2

    WkT = np.ascontiguousarray(Wk.T)
    MT = WkT @ Wq                             # M^T = Wk^T Wq  [D, D]
    ck = WkT @ bq                             # [D]
    gck = (g @ ck) * np.float32(1.0 / 32.0)   # [B]
    # M' = 32*M^T in DoubleRow pair layout [128, KP, 2, D]
    mtb = np.ascontiguousarray(
        (32.0 * MT).reshape(KP, 2, 128, D).transpose(2, 0, 1, 3)
    ).astype(NPF8)
    wvt = np.ascontiguousarray(
        Wv.T.reshape(KT, 128, D).transpose(1, 0, 2)
    ).astype(NPBF16)
    bv2f = np.broadcast_to(2.0 * bv, (128, D)).astype(np.float32).copy()
    idn = np.eye(128, dtype=np.float32).astype(NPBF16)

    n_chunks = R // CH
    f1h = f1.astype(NPBF16)
    f2h = f2.astype(NPBF16)
    in_maps = []
    for c in range(NCORES):
        sl = slice(c * R, (c + 1) * R)
        gs = g[sl]
        # g^T fp8: [n_chunks, 128(part=d%128), KP, 2, CH] (DoubleRow pairs)
        gtb = np.ascontiguousarray(
            gs.reshape(n_chunks, CH, KP, 2, 128).transpose(0, 4, 2, 3, 1)
        ).astype(NPF8)
        # f1|f2 rows packed per partition (bf16)
        pkf = np.concatenate(
            [
                f1h[sl].reshape(n_chunks, CH, D),
                f2h[sl].reshape(n_chunks, CH, D),
            ],
            axis=2,
        )
        gckb = np.ascontiguousarray(gck[sl].reshape(n_chunks, CH).T)
        in_maps.append(
            {
                "pkf": np.ascontiguousarray(pkf),
                "gtb": gtb,
                "gckb": gckb,
                "mtb": mtb,
                "wvt": wvt,
                "bv2f": bv2f,
                "idn": idn,
            }
        )
    return in_maps


def kernel(**inputs):
    from concourse.bass_utils import run_bass_kernel_spmd

    nc = get_compiled()
    in_maps = prep_inputs(**inputs)
    res = run_bass_kernel_spmd(nc, in_maps, core_ids=list(range(NCORES)))
    return np.concatenate([res.results[c]["out"] for c in range(NCORES)], axis=0)
